# revision 31
# baseline (speedup 1.0000x reference)
"""Trainium2 Bass kernel for int4-grouped-quantized linear (GPTQ-style).

out[8192, 11008] = x[8192, 4096] @ dequant(qweight, qzeros, scales)

Sharding: column-parallel over out_features N across 8 NeuronCores.

Device-side structure per core:
  - W dequant runs on DVE from a host byte-view of qweight: each nibble plane
    needs a single int ALU op (lo: &0xF, hi: >>4), then mult by s and subtract
    z*s using per-block [s | z*s] rows partition-broadcast via DMA. W's
    columns are ordered [all lo-nibbles | all hi-nibbles]; the host
    un-permutes output columns (pure reshape/transpose).
  - The GEMM runs in two N-half passes: pass A streams every output row-block
    against the lo-half of W (ready after only 32 lo-plane dequants), while
    the hi-half dequantizes in the background; pass B covers the hi-half.
    This halves the startup window in which the PE would starve on W.
  - x is pre-transposed/tiled on the host (layout only) into per-row-block
    [128 k, 128 t] stationary tiles; no on-device transposes. fp16 matmuls,
    fp32 PSUM accumulation.
"""

import sys

sys.path.insert(0, "/opt/trn_rl_repo")

from contextlib import ExitStack

import numpy as np

import concourse.bass as bass
from concourse import bacc
import concourse.tile as tile
from concourse import mybir
from concourse.bass_utils import run_bass_kernel_spmd

AOT = mybir.AluOpType
F16, I32, U8 = mybir.dt.float16, mybir.dt.int32, mybir.dt.uint8
F32 = mybir.dt.float32

T, K, N = 8192, 4096, 11008
NCORES = 8
NS = N // NCORES  # 1376 out cols per core
CS = NS // 8  # 172 packed int32 cols per core
CB = CS * 4  # 688 packed bytes per core (= NS/2, one nibble plane)
G = 32  # quant groups (group size 128 == one k-block)
KB = K // 128  # 32 k-blocks
QCH = 8  # k-blocks per qweight load chunk
NT = T // 128  # 64 output row-blocks
NA = 512  # pass-A device columns (one PSUM bank wide)
TAIL = CB - NA  # 176 lo-nibble tail columns handled in pass B
NB = NS - NA  # 864 pass-B device columns


def _body(ctx, tc, xtd, qwd, comb, outd):
    nc = tc.nc
    qpool = ctx.enter_context(tc.tile_pool(name="qwp", bufs=2))
    stpool = ctx.enter_context(tc.tile_pool(name="stage", bufs=2))
    wapool = ctx.enter_context(tc.tile_pool(name="wa", bufs=KB))
    wbpool = ctx.enter_context(tc.tile_pool(name="wb", bufs=KB))
    bcpool = ctx.enter_context(tc.tile_pool(name="bc", bufs=3))
    xpool = ctx.enter_context(tc.tile_pool(name="x", bufs=6))
    psapool = ctx.enter_context(tc.tile_pool(name="psa", bufs=4, space="PSUM"))
    psbpool = ctx.enter_context(tc.tile_pool(name="psb", bufs=2, space="PSUM"))
    opool = ctx.enter_context(tc.tile_pool(name="o", bufs=4))
    dpool = ctx.enter_context(tc.tile_pool(name="dummy", bufs=1))

    # ---- HAM warm-up: keep the PE busy from t=0 so the clock gate opens
    #      (K=8/8) before real matmuls start; results are discarded. ----
    dum = dpool.tile([128, 512], F16)
    nc.gpsimd.memset(dum[:], 0.0)
    dps = psbpool.tile([128, NB], F32, tag="ps")
    for _ in range(80):
        nc.tensor.matmul(dps[:, :512], dum[:, :128], dum[:], start=True, stop=True)

    # ---- prefetch the first x.T tiles ahead of the dequant DMA burst ----
    xt_pre = []
    for tg in range(6):
        xt_t = xpool.tile([128, KB * 128], F16, tag="xt")
        nc.gpsimd.dma_start(xt_t[:], xtd[tg * 128 : (tg + 1) * 128, :])
        xt_pre.append(xt_t)

    # ---- dequantize a device-column range of W: w = w4*s - z*s per k-block.
    #      comb rows hold [sA | zsA | sB | zsB] in device column order.
    #      Pass A = device cols [0, NA) (lo nibbles); pass B = [NA, NS)
    #      (lo-nibble tail + the whole hi plane, contiguous). ----
    def dequant_a():
        w_tiles = []
        for q in range(KB // QCH):
            qw_t = qpool.tile([128, QCH * CB], U8, tag="qw")
            nc.gpsimd.dma_start(
                qw_t[:].rearrange("p (b c) -> p b c", b=QCH),
                qwd[q * QCH * 128 : (q + 1) * QCH * 128, :].rearrange(
                    "(b p) c -> p b c", p=128
                ),
            )
            for i in range(QCH):
                b = q * QCH + i
                qw_b = qw_t[:, i * CB : (i + 1) * CB]
                bc = bcpool.tile([128, 2 * NA], F16, tag="bca")
                nc.gpsimd.dma_start(
                    bc[:], comb[b : b + 1, : 2 * NA].partition_broadcast(128)
                )
                wst = stpool.tile([128, NA], U8, tag="wsta")
                nc.vector.tensor_scalar(
                    wst[:], qw_b[:, :NA], 0xF, None, AOT.bitwise_and
                )
                w_t = wapool.tile([128, NA], F16)
                nc.vector.tensor_tensor(w_t[:], bc[:, :NA], wst[:], AOT.mult)
                nc.vector.tensor_tensor(w_t[:], w_t[:], bc[:, NA:], AOT.subtract)
                w_tiles.append(w_t)
        return w_tiles

    def dequant_b():
        w_tiles = []
        for q in range(KB // QCH):
            qw_t = qpool.tile([128, QCH * CB], U8, tag="qw")
            nc.gpsimd.dma_start(
                qw_t[:].rearrange("p (b c) -> p b c", b=QCH),
                qwd[q * QCH * 128 : (q + 1) * QCH * 128, :].rearrange(
                    "(b p) c -> p b c", p=128
                ),
            )
            for i in range(QCH):
                b = q * QCH + i
                qw_b = qw_t[:, i * CB : (i + 1) * CB]
                bc = bcpool.tile([128, 2 * NB], F16, tag="bcb")
                nc.gpsimd.dma_start(
                    bc[:], comb[b : b + 1, 2 * NA :].partition_broadcast(128)
                )
                wst = stpool.tile([128, NB], U8, tag="wstb")
                # cols [0, TAIL): lo nibbles of bytes NA..CB
                nc.vector.tensor_scalar(
                    wst[:, :TAIL], qw_b[:, NA:], 0xF, None, AOT.bitwise_and
                )
                # cols [TAIL, NB): hi nibbles of all bytes
                nc.vector.tensor_scalar(
                    wst[:, TAIL:], qw_b, 4, None, AOT.logical_shift_right
                )
                w_t = wbpool.tile([128, NB], F16)
                nc.vector.tensor_tensor(w_t[:], bc[:, :NB], wst[:], AOT.mult)
                nc.vector.tensor_tensor(w_t[:], w_t[:], bc[:, NB:], AOT.subtract)
                w_tiles.append(w_t)
        return w_tiles

    def gemm(w_tiles, col0, segs, pspool, first):
        width = sum(sz for _, sz in segs)
        for tg in range(NT):
            if first and tg < len(xt_pre):
                xt_t = xt_pre[tg]
            else:
                xt_t = xpool.tile([128, KB * 128], F16, tag="xt")
                nc.gpsimd.dma_start(xt_t[:], xtd[tg * 128 : (tg + 1) * 128, :])
            ps = pspool.tile([128, width], F32, tag="ps")
            for b in range(KB):
                lhs = xt_t[:, b * 128 : (b + 1) * 128]
                for off, sz in segs:
                    nc.tensor.matmul(
                        ps[:, off : off + sz],
                        lhs,
                        w_tiles[b][:, off : off + sz],
                        start=(b == 0),
                        stop=(b == KB - 1),
                    )
            ob = opool.tile([128, width], F16, tag=f"ob{col0}")
            if first:
                # ACT evacuates pass A so DVE can keep dequantizing pass-B W
                nc.scalar.copy(ob[:], ps[:])
            else:
                nc.vector.tensor_copy(ob[:], ps[:])
            r0 = tg * 128
            nc.gpsimd.dma_start(
                outd[r0 : r0 + 128, col0 : col0 + width], ob[:]
            )

    wA = dequant_a()
    wB = dequant_b()
    gemm(wA, 0, [(0, NA)], psapool, True)
    gemm(wB, NA, [(0, 512), (512, NB - 512)], psbpool, False)


def build_kernel():
    nc = bacc.Bacc("TRN2", target_bir_lowering=False, debug=False)
    xtd = nc.dram_tensor("xt", [NT * 128, KB * 128], F16, kind="ExternalInput").ap()
    qwd = nc.dram_tensor("qw", [K, CB], U8, kind="ExternalInput").ap()
    comb = nc.dram_tensor("comb", [G, 4 * CB], F16, kind="ExternalInput").ap()
    outd = nc.dram_tensor("out", [T, NS], F16, kind="ExternalOutput").ap()
    with tile.TileContext(nc) as tc, ExitStack() as ctx:
        _body(ctx, tc, xtd, qwd, comb, outd)
    nc.compile()
    return nc


_NC = None


def _get_nc():
    global _NC
    if _NC is None:
        _NC = build_kernel()
    return _NC


def _tile_xt(x):
    # x [T, K] -> xt [NT*128, KB*128] where
    # xt[tg*128 + p, b*128 + t] = x[tg*128 + t, b*128 + p]
    return np.ascontiguousarray(
        x.reshape(NT, 128, KB, 128).transpose(0, 3, 2, 1)
    ).reshape(NT * 128, KB * 128)


def _perm_cols(a):
    # reference col n = c*8 + j -> device col: lo nibbles (j=2k) first, hi after
    lead = a.shape[:-1]
    return np.ascontiguousarray(
        a.reshape(*lead, CS, 4, 2).transpose(*range(len(lead)), -1, -3, -2)
    ).reshape(*lead, NS)


def _unperm_out(o):
    # o [T, NS] device order -> reference column order
    return o.reshape(T, 2, CS, 4).transpose(0, 2, 3, 1).reshape(T, NS)


def make_in_maps(x, qweight, qzeros, scales):
    x = np.asarray(x, dtype=np.float16)
    qweight = np.asarray(qweight, dtype=np.int32)
    qzeros = np.asarray(qzeros, dtype=np.int32)
    scales = np.asarray(scales, dtype=np.float16)
    xt = _tile_xt(x)
    in_maps = []
    for c in range(NCORES):
        qw = np.ascontiguousarray(qweight[:, c * CS : (c + 1) * CS])
        qz = np.ascontiguousarray(qzeros[:, c * CS : (c + 1) * CS])
        sc = scales[:, c * NS : (c + 1) * NS]
        # group metadata in device (permuted) column order: [sA, zsA, sB, zsB]
        qz_u8 = qz.view(np.uint8).reshape(G, CB)
        z = np.concatenate([qz_u8 & 0xF, qz_u8 >> 4], axis=1).astype(np.float16)
        s_perm = _perm_cols(sc)
        zs = z * s_perm
        comb = np.concatenate(
            [s_perm[:, :NA], zs[:, :NA], s_perm[:, NA:], zs[:, NA:]], axis=1
        )
        in_maps.append(
            {
                "xt": xt,
                "qw": qw.view(np.uint8).reshape(K, CB),
                "comb": comb,
            }
        )
    return in_maps


def run(in_maps, **kwargs):
    return run_bass_kernel_spmd(
        _get_nc(), in_maps, core_ids=list(range(NCORES)), **kwargs
    )


def assemble(res):
    outs = [_unperm_out(res.results[c]["out"]) for c in range(NCORES)]
    return np.concatenate(outs, axis=1)


def kernel(x, qweight, qzeros, scales):
    res = run(make_in_maps(x, qweight, qzeros, scales))
    return assemble(res)
